# revision 1
# baseline (speedup 1.0000x reference)
"""Correlation (cost volume) kernel for Trainium2, 8-core data parallel.

Math (matches the reference):
  x1 = proj(input1), x2 = proj(input2)  (1x1 conv = per-pixel channel matmul)
  x2p = zero-pad(x2, 4 on each spatial side)
  out[b, di*9+dj, i, j] = sum_f x1[b,f,i,j] * x2p[b,f,i+di,j+dj] / sqrt(128)

Device strategy (per core, 4 batches each):
  - projections as [128c x 128f] matmuls (scale 128**-0.25 folded into W on
    both sides so the final /sqrt(128) is free)
  - correlation as banded matmuls: per output row i, three concurrent
    32-column-strip matmuls (tile_position col groups), stationary =
    projected x1 pixels [128c, 32j], moving = padded projected x2 window
    [128c, 9di x 40m] (40 = 32 + 8 slack for dj) -> PSUM band [96j, 360]
  - band tiles go back to DRAM; the per-partition diagonal de-skew
    out[..., dj] = band[..., (j%32)+dj] is done on the host (a gather the
    on-chip engines cannot express: per-partition offsets).

All matmul operands are bf16 (PSUM accumulates fp32).
"""
import math

import numpy as np
import ml_dtypes

import concourse.bass as bass
import concourse.bacc as bacc
import concourse.tile as tile
import concourse.mybir as mybir
from concourse.bass_utils import run_bass_kernel_spmd

B, C, H, W = 32, 128, 96, 96
NCORES = 8
BLOC = B // NCORES          # 4 batches per core
PATCH = 9
R = PATCH // 2              # 4
PH, PW = H + 2 * R, W + 2 * R  # 104 x 104 padded
NPIX = H * W                # 9216
PCHUNK = 384                # projection chunk: 4 image rows (384 px)
NCHUNK = NPIX // PCHUNK     # 24 exactly
WIN = 40                    # moving window per j-strip (32 + 8)
BAND = PATCH * WIN          # 360 band columns per output row
IGROUP = 8                  # output rows batched per SBUF band tile / DMA
OUT_DT = mybir.dt.bfloat16  # band DMA dtype (fp32 PSUM rounded once)

_cache: dict = {}


def _build_program():
    nc = bacc.Bacc(target_bir_lowering=False)
    bf = mybir.dt.bfloat16
    f32 = mybir.dt.float32

    x1d = nc.declare_dram_parameter("x1", [BLOC, C, NPIX], bf, isOutput=False)
    x2d = nc.declare_dram_parameter("x2", [BLOC, C, NPIX], bf, isOutput=False)
    wtd = nc.declare_dram_parameter("wt", [C, C], bf, isOutput=False)
    bd = nc.declare_dram_parameter("bias", [C, 1], f32, isOutput=False)
    bandd = nc.declare_dram_parameter(
        "band", [BLOC, H // IGROUP, H, IGROUP * BAND], OUT_DT, isOutput=True
    )

    with tile.TileContext(nc) as tc:
        with (
            tc.tile_pool(name="consts", bufs=1) as consts,
            tc.tile_pool(name="imgs", bufs=2) as imgs,
            tc.tile_pool(name="feats", bufs=2) as feats,
            tc.tile_pool(name="bands", bufs=4) as bands,
            tc.tile_pool(name="pps", bufs=3, space="PSUM") as pps,
            tc.tile_pool(name="bps", bufs=5, space="PSUM") as bps,
        ):
            wt = consts.tile([C, C], bf, tag="wt")
            nc.sync.dma_start(out=wt[:, :], in_=wtd[:, :])
            bias = consts.tile([C, 1], f32, tag="bias")
            nc.sync.dma_start(out=bias[:, :], in_=bd[:, :])

            ncopy = 0

            def copy(dst, src, add_bias):
                # split PSUM->SBUF copy load between DVE (4/9) and ACT (5/9)
                nonlocal ncopy
                ncopy += 1
                if ncopy % 9 < 4:
                    if add_bias:
                        nc.vector.tensor_scalar_add(dst, src, bias[:, :])
                    else:
                        nc.vector.tensor_copy(dst, src)
                else:
                    if add_bias:
                        nc.scalar.activation(
                            dst, src, mybir.ActivationFunctionType.Identity,
                            bias=bias[:, :],
                        )
                    else:
                        nc.scalar.copy(dst, src)

            for b in range(BLOC):
                x1t = imgs.tile([C, NPIX], bf, tag="x1")
                nc.sync.dma_start(out=x1t[:, :], in_=x1d[b, :, :])
                x2t = imgs.tile([C, NPIX], bf, tag="x2")
                nc.sync.dma_start(out=x2t[:, :], in_=x2d[b, :, :])

                y1 = feats.tile([C, NPIX], bf, tag="y1")
                z2 = feats.tile([C, PH * PW], bf, tag="z2")
                z2v = z2[:, :].rearrange("c (r w) -> c r w", w=PW)
                # zero the pad frame (gpsimd; interior is fully overwritten)
                nc.gpsimd.memset(z2v[:, 0:R, :], 0.0)
                nc.gpsimd.memset(z2v[:, R + H:PH, :], 0.0)
                nc.gpsimd.memset(z2v[:, R:R + H, 0:R], 0.0)
                nc.gpsimd.memset(z2v[:, R:R + H, R + W:PW], 0.0)

                for k in range(NCHUNK):
                    sl = bass.ts(k, PCHUNK)
                    p1 = pps.tile([C, PCHUNK], f32, tag="pp")
                    nc.tensor.matmul(p1[:, :], wt[:, :], x1t[:, sl],
                                     start=True, stop=True)
                    copy(y1[:, sl], p1[:, :], True)
                    p2 = pps.tile([C, PCHUNK], f32, tag="pp")
                    nc.tensor.matmul(p2[:, :], wt[:, :], x2t[:, sl],
                                     start=True, stop=True)
                    p2v = p2[:, :].rearrange("c (r w) -> c r w", w=W)
                    copy(z2v[:, R + 4 * k:R + 4 * k + 4, R:R + W], p2v, True)

                y1v = y1[:, :].rearrange("c (i j) -> c i j", j=W)
                for g in range(H // IGROUP):
                    bt = bands.tile([H, IGROUP * BAND], OUT_DT, tag="bt")
                    for s in range(IGROUP):
                        i = IGROUP * g + s
                        pb = bps.tile([C, BAND], f32, tag="pb")
                        for jb in range(3):
                            nc.tensor.matmul(
                                pb[32 * jb:32 * jb + 32, :],
                                y1v[:, i, 32 * jb:32 * jb + 32],
                                z2v[:, i:i + PATCH, 32 * jb:32 * jb + WIN],
                                start=True, stop=True,
                                tile_position=(0, 32 * jb),
                            )
                        copy(bt[:, bass.ts(s, BAND)], pb[0:H, :], False)
                    nc.sync.dma_start(out=bandd[b, g, :, :], in_=bt[:, :])

    nc.compile()
    return nc


def kernel(input1, input2, proj_w, proj_b):
    if "nc" not in _cache:
        _cache["nc"] = _build_program()
    nc = _cache["nc"]

    s = float(C) ** -0.25  # applied to both projections -> 1/sqrt(C) total
    wt = np.ascontiguousarray((proj_w.astype(np.float64) * s).T).astype(
        ml_dtypes.bfloat16
    )
    bias = (proj_b.astype(np.float64) * s).astype(np.float32).reshape(C, 1)

    in_maps = []
    for k in range(NCORES):
        sl = slice(BLOC * k, BLOC * (k + 1))
        in_maps.append({
            "x1": np.ascontiguousarray(input1[sl]).reshape(BLOC, C, NPIX)
                    .astype(ml_dtypes.bfloat16),
            "x2": np.ascontiguousarray(input2[sl]).reshape(BLOC, C, NPIX)
                    .astype(ml_dtypes.bfloat16),
            "wt": wt,
            "bias": bias,
        })

    res = run_bass_kernel_spmd(nc, in_maps, list(range(NCORES)))

    # host de-skew: out[b, di*9+dj, i, j] = band[b, i, j, di, (j%32)+dj]
    j = np.arange(W)
    idx = (j % 32)[:, None] + np.arange(PATCH)[None, :]        # [96, 9]
    idx6 = np.broadcast_to(idx[None, None, :, None, :],
                           (BLOC, H, W, PATCH, PATCH))
    outs = []
    for k in range(NCORES):
        band = np.asarray(res.results[k]["band"], dtype=np.float32)
        # [BLOC, 24, j, s, di, m] -> [BLOC, i, j, di, m]
        band = band.reshape(BLOC, H // IGROUP, H, IGROUP, PATCH, WIN)
        band = band.transpose(0, 1, 3, 2, 4, 5).reshape(BLOC, H, W, PATCH, WIN)
        sel = np.take_along_axis(band, idx6, axis=-1)          # [b,i,j,di,dj]
        outs.append(sel.transpose(0, 3, 4, 1, 2).reshape(BLOC, PATCH * PATCH, H, W))
    return np.concatenate(outs, axis=0)



# revision 4
# speedup vs baseline: 1.4326x; 1.4326x over previous
"""Correlation (cost volume) kernel for Trainium2, 8-core data parallel.

Math (matches the reference):
  x1 = proj(input1), x2 = proj(input2)  (1x1 conv = per-pixel channel matmul)
  x2p = zero-pad(x2, 4 on each spatial side)
  out[b, di*9+dj, i, j] = sum_f x1[b,f,i,j] * x2p[b,f,i+di,j+dj] / sqrt(128)

Key algebra: <W x1 + b, W x2 + b> = x1 . (M x2 + W^T b) + b . (W x2 + b)
with M = W^T W. With the harness' b = 0 the second term vanishes, so only
ONE on-chip projection is needed (z2 = M x2, with 1/sqrt(128) folded in)
and x1 is used raw.

Device strategy (per core, 4 batches each):
  - z2 = M @ x2 as [128c x 128c] matmuls over 384-pixel chunks, written
    into a zero-padded [128, 104, 104] SBUF image.
  - correlation as 2D-block matmuls: stationary = raw x1 pixel block
    [128c, 8x16 px] (all 128 PE columns used), moving = padded z2 window
    [128c, 16x24] = 384 columns -> PSUM [128px, 384].  Each moving column
    is shared by up to 81 (di,dj) outputs, so the correlation costs only
    3 moving columns per pixel (vs 11.25 for row-banded strips).
  - PSUM band tiles -> SBUF (bf16) -> DRAM; the per-pixel diagonal
    de-skew out[p, di, dj] = band[p, 24*(r+di) + (c+dj)] is a pure
    numpy as_strided view on the host (per-partition offsets cannot be
    expressed by on-chip compute-engine access patterns).

All matmul operands are bf16 (PSUM accumulates fp32).
"""
import math

import numpy as np
import ml_dtypes

import concourse.bass as bass
import concourse.bacc as bacc
import concourse.tile as tile
import concourse.mybir as mybir
from concourse.bass_utils import run_bass_kernel_spmd

B, C, H, W = 32, 128, 96, 96
NCORES = 8
BLOC = B // NCORES          # 4 batches per core
PATCH = 9
R = PATCH // 2              # 4
PH, PW = H + 2 * R, W + 2 * R  # 104 x 104 padded
NPIX = H * W                # 9216
PCHUNK = 384                # projection chunk: 4 image rows (384 px)
NCHUNK = NPIX // PCHUNK     # 24 exactly
BH, BW = 8, 16              # correlation block: 8 x 16 pixels = 128
MH, MW = BH + 2 * R, BW + 2 * R  # 16 x 24 moving window
MCOLS = MH * MW             # 384 moving columns per block
NBI, NBJ = H // BH, W // BW  # 12 x 6 blocks per image
OUT_DT = mybir.dt.bfloat16  # band DMA dtype (fp32 PSUM rounded once)

_cache: dict = {}


def _build_program():
    nc = bacc.Bacc(target_bir_lowering=False)
    bf = mybir.dt.bfloat16
    f32 = mybir.dt.float32

    x1d = nc.declare_dram_parameter("x1", [BLOC, C, NPIX], bf, isOutput=False)
    x2d = nc.declare_dram_parameter("x2", [BLOC, C, NPIX], bf, isOutput=False)
    mtd = nc.declare_dram_parameter("mt", [C, C], bf, isOutput=False)
    bd = nc.declare_dram_parameter("bias", [C, 1], f32, isOutput=False)
    bandd = nc.declare_dram_parameter(
        "band", [BLOC, NBI, C, NBJ * MCOLS], OUT_DT, isOutput=True
    )

    with tile.TileContext(nc) as tc:
        with (
            tc.tile_pool(name="consts", bufs=1) as consts,
            tc.tile_pool(name="imgs", bufs=2) as imgs,
            tc.tile_pool(name="feats", bufs=2) as feats,
            tc.tile_pool(name="bands", bufs=3) as bands,
            tc.tile_pool(name="pps", bufs=2, space="PSUM") as pps,
            tc.tile_pool(name="bps", bufs=4, space="PSUM") as bps,
        ):
            mt = consts.tile([C, C], bf, tag="mt")
            nc.sync.dma_start(out=mt[:, :], in_=mtd[:, :])
            bias = consts.tile([C, 1], f32, tag="bias")
            nc.sync.dma_start(out=bias[:, :], in_=bd[:, :])

            ncopy = 0

            def copy(dst, src, add_bias):
                # split PSUM->SBUF copy load between DVE (~47%) and ACT
                nonlocal ncopy
                ncopy += 1
                if ncopy % 15 < 7:
                    if add_bias:
                        nc.vector.tensor_scalar_add(dst, src, bias[:, :])
                    else:
                        nc.vector.tensor_copy(dst, src)
                else:
                    if add_bias:
                        nc.scalar.activation(
                            dst, src, mybir.ActivationFunctionType.Identity,
                            bias=bias[:, :],
                        )
                    else:
                        nc.scalar.copy(dst, src)

            for b in range(BLOC):
                x1t = imgs.tile([C, NPIX], bf, tag="x1")
                nc.sync.dma_start(out=x1t[:, :], in_=x1d[b, :, :])
                x2t = imgs.tile([C, NPIX], bf, tag="x2")
                nc.sync.dma_start(out=x2t[:, :], in_=x2d[b, :, :])

                z2 = feats.tile([C, PH * PW], bf, tag="z2")
                z2v = z2[:, :].rearrange("c (r w) -> c r w", w=PW)
                # zero the pad frame (gpsimd; interior is fully overwritten)
                nc.gpsimd.memset(z2v[:, 0:R, :], 0.0)
                nc.gpsimd.memset(z2v[:, R + H:PH, :], 0.0)
                nc.gpsimd.memset(z2v[:, R:R + H, 0:R], 0.0)
                nc.gpsimd.memset(z2v[:, R:R + H, R + W:PW], 0.0)

                # z2 interior = M @ x2 (+ W^T b), in 4-image-row chunks
                for k in range(NCHUNK):
                    p2 = pps.tile([C, PCHUNK], f32, tag="pp")
                    nc.tensor.matmul(p2[:, :], mt[:, :], x2t[:, bass.ts(k, PCHUNK)],
                                     start=True, stop=True)
                    p2v = p2[:, :].rearrange("c (r w) -> c r w", w=W)
                    copy(z2v[:, R + 4 * k:R + 4 * k + 4, R:R + W], p2v, True)

                # x1 is host-pre-blocked: [c, bi, bj, 128] with p = 16*r + c
                x1v = x1t[:, :].rearrange("c (bi bj p) -> c bi bj p",
                                          bj=NBJ, p=BH * BW)
                for bi in range(NBI):
                    bt = bands.tile([C, NBJ * MCOLS], OUT_DT, tag="bt")
                    for bj in range(NBJ):
                        pb = bps.tile([C, MCOLS], f32, tag="pb")
                        nc.tensor.matmul(
                            pb[:, :],
                            x1v[:, bi, bj, :],
                            z2v[:, BH * bi:BH * bi + MH, BW * bj:BW * bj + MW],
                            start=True, stop=True,
                        )
                        copy(bt[:, bass.ts(bj, MCOLS)], pb[:, :], False)
                    nc.sync.dma_start(out=bandd[b, bi, :, :], in_=bt[:, :])

    nc.compile()
    return nc


def kernel(input1, input2, proj_w, proj_b):
    if "nc" not in _cache:
        _cache["nc"] = _build_program()
    nc = _cache["nc"]

    w64 = np.asarray(proj_w, dtype=np.float64)
    b64 = np.asarray(proj_b, dtype=np.float64)
    s = 1.0 / math.sqrt(C)
    m = (w64.T @ w64) * s          # symmetric: stationary-transpose safe
    mt = np.ascontiguousarray(m).astype(ml_dtypes.bfloat16)
    bias = (w64.T @ b64 * s).astype(np.float32).reshape(C, 1)

    # pre-block x1 so each 8x16 correlation block is a contiguous
    # single-free-dim stationary slice: [b, c, bi, bj, 16*r + c']
    x1b = (np.asarray(input1)
           .reshape(B, C, NBI, BH, NBJ, BW)
           .transpose(0, 1, 2, 4, 3, 5)
           .reshape(B, C, NPIX)
           .astype(ml_dtypes.bfloat16))

    in_maps = []
    for k in range(NCORES):
        sl = slice(BLOC * k, BLOC * (k + 1))
        in_maps.append({
            "x1": np.ascontiguousarray(x1b[sl]),
            "x2": np.ascontiguousarray(input2[sl]).reshape(BLOC, C, NPIX)
                    .astype(ml_dtypes.bfloat16),
            "mt": mt,
            "bias": bias,
        })

    res = run_bass_kernel_spmd(nc, in_maps, list(range(NCORES)))

    # host de-skew: out[b, di*9+dj, 8*bi+r, 16*bj+c]
    #             = band[b, bi, 16*r+c, bj, 24*(r+di) + (c+dj)]
    outs = []
    for k in range(NCORES):
        band = np.asarray(res.results[k]["band"])
        v = band.reshape(BLOC, NBI, BH, BW, NBJ, MH, MW)
        st = v.strides
        sel = np.lib.stride_tricks.as_strided(
            v,
            shape=(BLOC, PATCH, PATCH, NBI, BH, NBJ, BW),
            strides=(st[0], st[5], st[6], st[1], st[2] + st[5], st[4],
                     st[3] + st[6]),
        )
        outs.append(sel.astype(np.float32).reshape(BLOC, PATCH * PATCH, H, W))
    out = np.concatenate(outs, axis=0)

    if np.any(b64 != 0.0):
        # general-bias correction: b . pad(W x2 + b) term (zero in harness)
        y2 = np.einsum("fc,bchw->bfhw", w64, np.asarray(input2, np.float64))
        t = (np.einsum("f,bfhw->bhw", b64, y2 + b64[None, :, None, None]) * s)
        tp = np.pad(t, ((0, 0), (R, R), (R, R)))
        for di in range(PATCH):
            for dj in range(PATCH):
                out[:, di * PATCH + dj] += tp[:, di:di + H, dj:dj + W].astype(
                    np.float32)
    return out
